# revision 90
# baseline (speedup 1.0000x reference)
"""Deformable PSROI pooling (group_size=1, num_classes=1) on 8 trn2 NeuronCores.

Strategy ("map sweep"):
  out[n, c, ph, pw] = sum_{y,x} KY[bin, y] * KX[bin, x] * data[b, c, y, x]
where KX/KY are per-bin bilinear "hat" weight profiles (sums over the 4x4
sample grid, with sample masks and 1/count folded in).  Each core holds a
slice of one batch's feature map in SBUF in [x(partitions), (y, c)] layout;
for each feature row y it issues one fp16 TensorE matmul
    psum[c, bins] += map_row[x, c].T @ W_y[x, bins]
accumulating bins in PSUM 512-column "generations" (bins sorted by first
active row), with host-precomputed W streamed per generation.

Everything is fp16 (weights, map, staged output; fp32 PSUM accumulate),
which quarters the fp32 PE pass count and halves HBM traffic.  A short
burst of dummy matmuls at the start of each rep keeps ~3.4us of continuous
PE activity so the HAM clock gate lifts the PE from 1.2GHz to 2.4GHz before
the real stream begins (worth ~1.5x end to end).

Sharding: bins are split by (batch, ylo-quantile) into 8 equal-count shards
(quantile keeps each core's map read to a ~48-row window).  The compiled
program is shared by all cores; all per-core variation lives in the input
tensors (map slice, W stream).  Cross-core schedule alignment is
per-generation, with the map supplied as per-generation row segments so each
core anchors a generation at its own starting row; matmul column windows are
the cross-core envelope, with zero weights padding inactive slots.

Probed dead ends kept for the record: switching PE tile_position between
matmuls hangs the exec unit with this toolchain, and the (64,128) row-tiled
mode runs ~2x slower per matmul, so narrow-K x-window compression of W
(quad/half streams) loses to the dense K=128 sweep even though it moves
~2.5x fewer W bytes.  NQ/WW plumbing for those variants remains (KNQ env).
"""
import os
import sys
import time

import numpy as np

sys.path.insert(0, "/opt/trn_rl_repo")

SPATIAL_SCALE = np.float32(0.0625)
POOLED = 7
SAMPLES = 4
TRANS_STD = np.float32(0.1)
B, C, H, W = 2, 128, 128, 128
NCORES = 8
GEN_COLS = 512
# "f32" (exact, 4-pass PE), "f16" (1-pass, ~1e-3, half the DMA bytes)
DT_MODE = os.environ.get("KDT", "f16")

f32 = np.float32
YSENT = 10 ** 6  # sentinel ylo for bins with all-zero weights
# x-windows: two 64-row halves [0,64) and [64,128), both presented to the PE
# at partitions [0,64) (the map tile holds the upper half relocated), so every
# matmul is a uniform K=64 tile_size=(64,128) at tile_position (0,0).
# Switching tile_position between matmuls hangs the exec unit (probed), so a
# single uniform tile is required.
NQ = int(os.environ.get("KNQ", "1"))
WW = 128 // NQ
# contraction uses KR partitions (one always-zero pad row when WW<128):
# partition size >64 rounds the PE tile_size up to the standard (128,128)
# mode.  The (64,128) row-tiled mode measured ~2x slower per matmul (weight
# load no longer overlaps), and switching tile_position between matmuls
# hangs the exec unit.
KR = min(WW + 1, 128)
WIDEN = os.environ.get("KWIDEN", "0") == "1"
# dummy matmuls at rep start: ~3.4us of continuous PE activity trips the HAM
# clock gate from 1.2GHz to 2.4GHz before the real stream begins
WARM = int(os.environ.get("KWARM", "8"))


# ----------------------------------------------------------------------------
# host planning
# ----------------------------------------------------------------------------

def _bin_params(rois, offset):
    """Exact float32 emulation of the reference coordinate math.

    Returns per-bin (N*49) arrays: batch, dense hat profiles kx/ky [nb, 128]
    (ky has 1/count folded in), y-support [ylo, yhi], x-support class.
    """
    N = rois.shape[0]
    P, S = POOLED, SAMPLES
    rois = rois.astype(f32)
    offset = offset.astype(f32)

    batch_ind = rois[:, 0].astype(np.int32)
    roi_sw = np.round(rois[:, 1]) * SPATIAL_SCALE - f32(0.5)
    roi_sh = np.round(rois[:, 2]) * SPATIAL_SCALE - f32(0.5)
    roi_ew = np.round(rois[:, 3] + f32(1.0)) * SPATIAL_SCALE - f32(0.5)
    roi_eh = np.round(rois[:, 4] + f32(1.0)) * SPATIAL_SCALE - f32(0.5)
    roi_w = np.maximum(roi_ew - roi_sw, f32(0.1))
    roi_h = np.maximum(roi_eh - roi_sh, f32(0.1))
    bin_w = roi_w / f32(P)
    bin_h = roi_h / f32(P)
    sub_w = bin_w / f32(S)
    sub_h = bin_h / f32(S)

    pidx = np.arange(P, dtype=f32)
    trans_x = offset[:, 0] * TRANS_STD          # [N, 7(ph), 7(pw)]
    trans_y = offset[:, 1] * TRANS_STD
    pw = pidx[None, None, :]
    ph = pidx[None, :, None]
    wstart = pw * bin_w[:, None, None] + roi_sw[:, None, None] + trans_x * roi_w[:, None, None]
    hstart = ph * bin_h[:, None, None] + roi_sh[:, None, None] + trans_y * roi_h[:, None, None]

    sidx = np.arange(S, dtype=f32)
    w_s = wstart[..., None] + sidx * sub_w[:, None, None, None]     # [N,7,7,4]
    h_s = hstart[..., None] + sidx * sub_h[:, None, None, None]
    mask_w = (w_s >= f32(-0.5)) & (w_s <= f32(W) - f32(0.5))
    mask_h = (h_s >= f32(-0.5)) & (h_s <= f32(H) - f32(0.5))
    wc = np.clip(w_s, f32(0.0), f32(W - 1))
    hc = np.clip(h_s, f32(0.0), f32(H - 1))

    cnt = (mask_h.sum(-1) * mask_w.sum(-1)).astype(f32)             # [N,7,7]
    inv = np.where(cnt > 0, f32(1.0) / np.maximum(cnt, f32(1.0)), f32(0.0))

    nb = N * P * P
    wc = wc.reshape(nb, S)
    hc = hc.reshape(nb, S)
    mask_w = mask_w.reshape(nb, S)
    mask_h = mask_h.reshape(nb, S)
    inv = inv.reshape(nb)

    xg = np.arange(W, dtype=np.float64)
    kx = np.zeros((nb, W), np.float64)
    ky = np.zeros((nb, H), np.float64)
    for s in range(S):
        kx += mask_w[:, s, None] * np.maximum(0.0, 1.0 - np.abs(wc[:, s, None].astype(np.float64) - xg))
        ky += mask_h[:, s, None] * np.maximum(0.0, 1.0 - np.abs(hc[:, s, None].astype(np.float64) - xg))
    ky *= inv[:, None]
    kx = kx.astype(f32)
    ky = ky.astype(f32)

    ky_nz = ky != 0
    has_y = ky_nz.any(axis=1)
    ylo = np.where(has_y, ky_nz.argmax(axis=1), YSENT).astype(np.int64)
    yhi = np.where(has_y, H - 1 - ky_nz[:, ::-1].argmax(axis=1), -YSENT).astype(np.int64)

    kx_nz = kx != 0
    has_x = kx_nz.any(axis=1)
    xlo = np.where(has_x, kx_nz.argmax(axis=1), 0).astype(np.int64)
    xhi = np.where(has_x, W - 1 - kx_nz[:, ::-1].argmax(axis=1), -1).astype(np.int64)

    ok = has_y & has_x
    ylo = np.where(ok, ylo, YSENT)
    yhi = np.where(ok, yhi, -YSENT)
    # per-bin active x-windows: window q active iff kx[WW*q : WW*(q+1)] != 0
    qact = np.zeros((nb, NQ), bool)
    for q in range(NQ):
        qact[:, q] = ok & (xlo <= WW * q + WW - 1) & (xhi >= WW * q)

    batch = np.repeat(batch_ind, P * P)
    return batch, kx, ky, ylo, yhi, ok, qact


def _plan(rois, offset):
    batch, kx, ky, ylo, yhi, ok, qact = _bin_params(rois, offset)
    G = GEN_COLS

    # shard bins: (batch, ylo-quantile) -> 8 shards with equal-ish counts
    shards = []
    for b in range(B):
        ids = np.where(batch == b)[0]
        ids = ids[np.lexsort((yhi[ids], ylo[ids]))]
        q = NCORES // B
        shards.extend(ids[int(len(ids) * i / q):int(len(ids) * (i + 1) / q)]
                      for i in range(q))
    assert len(shards) == NCORES

    # expand bins -> (bin, quad) pairs; a straddling bin occupies one slot in
    # each of its (at most 2) active quads, summed on the host at unshard.
    # Pairs are kept in (ylo, yhi) order within each quad queue.
    pair_q_lists = []   # [ci][q] -> bin ids in (ylo, yhi) order
    for ci in range(NCORES):
        ids = shards[ci]
        pair_q_lists.append([ids[qact[ids, q]] for q in range(NQ)])

    # per-shard absolute row window
    row_start = np.zeros(NCORES, np.int64)
    extents = []
    for ids in shards:
        real = ids[ylo[ids] < YSENT]
        if len(real):
            extents.append((int(ylo[real].min()), int(yhi[real].max())))
        else:
            extents.append((0, 0))
    rstar = max(b_ - a_ + 1 for a_, b_ in extents)
    rstar = min(H, -(-rstar // 8) * 8)
    # anchor at each core's own extent start (map rows past H are zero-padded)
    # so per-gen windows align across cores in relative-row space
    for ci, (a_, b_) in enumerate(extents):
        row_start[ci] = a_

    # generation layout with SHARED window-run offsets.  Each core's pairs
    # form one (ylo, yhi)-sorted stream; all cores advance in lockstep by k
    # pairs per gen — same stream index means the same quantile position, so
    # window profiles align across cores once each (core, gen) re-anchors at
    # its own first row (the map is supplied as per-gen row segments).
    # Per (gen, window) capacity R_gq = cross-core max of pairs taken; cores
    # with fewer pad with dead slots.
    pair_bin, pair_quad, qcums = [], [], []
    for ci in range(NCORES):
        ids = shards[ci]
        pb = np.repeat(ids, qact[ids].sum(axis=1))
        pq = np.concatenate([np.nonzero(qact[i])[0] for i in ids]) \
            if len(ids) else np.zeros(0, np.int64)
        pair_bin.append(pb.astype(np.int64))
        pair_quad.append(pq.astype(np.int64))
        qcums.append(np.concatenate(
            [np.zeros((NQ, 1), np.int64),
             np.cumsum(pq[None, :] == np.arange(NQ)[:, None], axis=1)], axis=1)
            if len(pq) else np.zeros((NQ, 1), np.int64))
    totals = np.array([len(p) for p in pair_bin], np.int64)
    ptr = np.zeros(NCORES, np.int64)
    gen_caps = []       # [g][q] -> R_gq
    gen_k = []          # [g] -> pairs taken per core

    def quad_counts(k):
        n = np.zeros((NCORES, NQ), np.int64)
        for ci in range(NCORES):
            e = min(int(ptr[ci]) + k, int(totals[ci]))
            n[ci] = qcums[ci][:, e] - qcums[ci][:, ptr[ci]]
        return n

    while (ptr < totals).any():
        lo_k, hi_k = 1, int((totals - ptr).max())
        while lo_k < hi_k:     # largest k with sum_q max_ci counts <= G
            mid = (lo_k + hi_k + 1) // 2
            if quad_counts(mid).max(axis=0).sum() <= G:
                lo_k = mid
            else:
                hi_k = mid - 1
        n = quad_counts(lo_k)
        gen_caps.append([int(n[:, q].max()) for q in range(NQ)])
        gen_k.append(lo_k)
        ptr = np.minimum(ptr + lo_k, totals)
    ngens = len(gen_caps)
    gen_cols = [sum(R) for R in gen_caps]
    coff = np.concatenate([[0], np.cumsum(gen_cols)]).astype(np.int64)
    nslots = int(coff[-1])

    # per-(core, gen) row anchor + per-gen map segments
    t0 = np.zeros((NCORES, ngens), np.int64)
    span = np.zeros((NCORES, ngens), np.int64)
    runs = {}
    slot_bin = []      # per core: bin id for each slot, -1 for pad slots
    for ci in range(NCORES):
        rs = int(row_start[ci])
        p = 0
        slots = np.full(nslots, -1, np.int64)
        for g, R in enumerate(gen_caps):
            e = min(p + gen_k[g], int(totals[ci]))
            gb = pair_bin[ci][p:e]
            gq = pair_quad[ci][p:e]
            p = e
            if len(gb) == 0:
                continue
            real = gb[ylo[gb] < YSENT]
            if len(real) == 0:
                continue
            a_ = int(ylo[real].min()) - rs
            t0[ci, g] = a_
            span[ci, g] = int(yhi[real].max()) - rs - a_ + 1
            roff = 0
            for q in range(NQ):
                m = gq == q
                take = int(m.sum())
                if take > 0:
                    sel = gb[m]
                    slots[coff[g] + roff:coff[g] + roff + take] = sel
                    runs[(ci, g, q)] = (roff, ylo[sel] - rs - a_,
                                        yhi[sel] - rs - a_, take)
                roff += R[q]
        slot_bin.append(slots)
    steps = [int(span[:, g].max()) for g in range(ngens)]
    seg_off = np.concatenate([[0], np.cumsum(steps)]).astype(np.int64)
    srows = int(seg_off[-1])

    # shared schedule: per (gen, step, quad) the column envelope and the W
    # stream cursor.  W layout per gen: the 4 quad streams stacked in
    # partition quarters over one rectangular [128, Lg] block (one DMA).
    sched = []          # (g, s, q, cl, ch, col0, first, last)
    gen_ltot = []
    for g, R in enumerate(gen_caps):
        S_g = steps[g]
        qoff = np.concatenate([[0], np.cumsum(R)]).astype(np.int64)
        cur = 0
        entries = []
        widened = set()
        for s in range(S_g):
            for q in range(NQ):
                cl, ch = YSENT, 0
                for ci in range(NCORES):
                    r = runs.get((ci, g, q))
                    if r is None:
                        continue
                    roff, yl, yh, take = r
                    # shared step: row rel T[g]+s for every core
                    hi = int(np.count_nonzero(yl <= s))
                    live = np.nonzero(yh >= s)[0]
                    lo = int(live[0]) if len(live) else hi
                    if hi > lo:
                        cl = min(cl, roff + lo)
                        ch = max(ch, roff + hi)
                if ch > cl:
                    if WIDEN and q not in widened:
                        # first entry of this window covers the whole run so
                        # every slot (incl. pads) is written exactly once
                        # (pad slots otherwise hold stale PSUM, masked on the
                        # host; only the strict simulator needs the cover)
                        widened.add(q)
                        cl, ch = int(qoff[q]), int(qoff[q + 1])
                    entries.append((g, s, q, cl, ch, cur))
                    cur += ch - cl
        gen_ltot.append(cur)
        for i, e in enumerate(entries):
            sched.append(e + (i == 0, i == len(entries) - 1))
    gen_woff = np.concatenate([[0], np.cumsum(gen_ltot)]).astype(np.int64)
    wtot = int(gen_woff[-1])

    meta = dict(rstar=int(rstar), ngens=int(ngens), nslots=int(nslots),
                sched=tuple(sched), wtot=wtot, srows=srows,
                steps=tuple(steps),
                seg_off=tuple(int(x) for x in seg_off),
                gen_ltot=tuple(gen_ltot), gen_cols=tuple(gen_cols),
                coff=tuple(int(x) for x in coff),
                gen_woff=tuple(int(x) for x in gen_woff))
    return dict(meta=meta, slot_bin=slot_bin, runs=runs, t0=t0,
                row_start=row_start,
                kx=kx, ky=ky, ylo=ylo, ok=ok)


def _build_inputs(plan, data):
    meta = plan["meta"]
    sched, srows, wtot = meta["sched"], meta["srows"], meta["wtot"]
    gen_woff, coff, seg_off = meta["gen_woff"], meta["coff"], meta["seg_off"]
    kx, ky, runs = plan["kx"], plan["ky"], plan["runs"]
    t0 = plan["t0"]
    data_perm = np.ascontiguousarray(data.transpose(0, 3, 2, 1))  # [B, W(x), H(y), C]

    np_dt = np.float16 if DT_MODE == "f16" else f32
    in_maps = []
    for ci in range(NCORES):
        sbin = plan["slot_bin"][ci]
        b = ci // (NCORES // B)
        rs = int(plan["row_start"][ci])
        # per-gen map row segments; window q's x-rows at partitions [0, WW)
        mp = np.zeros((KR, NQ, srows, C), np_dt)
        for g, S_g in enumerate(meta["steps"]):
            if S_g == 0:
                continue
            y0 = rs + int(t0[ci, g])
            y1 = min(H, y0 + S_g)
            if y1 <= y0:
                continue
            for q in range(NQ):
                mp[:WW, q, seg_off[g]:seg_off[g] + (y1 - y0), :] = \
                    data_perm[b, WW * q:WW * (q + 1), y0:y1, :]
        wbuf = np.zeros((KR, max(wtot, 8)), np_dt)
        for (g, s, q, cl, ch, col0, first, last) in sched:
            r = runs.get((ci, g, q))
            if r is None:
                continue
            y = rs + int(t0[ci, g]) + s
            if y >= H:
                continue
            roff, yl, yh, take = r
            # only this core's own window-q run; envelope overhang stays zero
            j0, j1 = max(cl, roff), min(ch, roff + take)
            if j1 <= j0:
                continue
            sel = sbin[coff[g] + j0:coff[g] + j1]
            vals = kx[sel, WW * q:WW * (q + 1)] * ky[sel, y][:, None]  # [n, WW]
            c0 = gen_woff[g] + col0 + (j0 - cl)
            wbuf[:WW, c0:c0 + (j1 - j0)] = vals.T
        in_maps.append({"mp": np.ascontiguousarray(mp.reshape(KR, NQ * srows * C)),
                        "w": wbuf})
    return in_maps


# ----------------------------------------------------------------------------
# device program
# ----------------------------------------------------------------------------

def _split_drains(nc, mybir, bass_rust):
    for f_ in nc.m.functions:
        for blk in f_.blocks:
            newlist = []
            for ins in blk.instructions:
                wts = list(ins.sync_info.on_wait) if ins.sync_info else []
                if len(wts) > 1 and type(ins).__name__ == "InstDrain":
                    for j, wx in enumerate(wts[1:]):
                        nop = mybir.InstNoOp(name=f"splitw_{id(ins)}_{j}", ins=[], outs=[])
                        nop.engine = ins.engine
                        nop.sync_info = bass_rust.SyncInfo(on_wait=[wx], on_update=[])
                        newlist.append(nop)
                    ins.sync_info.on_wait = wts[:1]
                newlist.append(ins)
            blk.instructions = newlist


def _build_program(meta, rep=1):
    import concourse.bacc as bacc
    import concourse.mybir as mybir
    import bass_rust
    from concourse.tile import TileContext

    ngens, nslots = meta["ngens"], meta["nslots"]
    sched, wtot, srows = meta["sched"], meta["wtot"], meta["srows"]
    steps, seg_off = meta["steps"], meta["seg_off"]
    gen_woff, gen_ltot = meta["gen_woff"], meta["gen_ltot"]
    dt = mybir.dt.float16 if DT_MODE == "f16" else mybir.dt.float32
    out_dt = mybir.dt.float16 if DT_MODE == "f16" else mybir.dt.float32
    G = GEN_COLS

    nc = bacc.Bacc()
    mp = nc.declare_dram_parameter("mp", [KR, NQ * srows * C], dt, isOutput=False)
    w = nc.declare_dram_parameter("w", [KR, max(wtot, 8)], dt, isOutput=False)
    o = nc.declare_dram_parameter("o", [128, nslots], out_dt, isOutput=True)

    with TileContext(nc) as tc:
        with (
            tc.tile_pool(name="const", bufs=1) as constp,
            tc.tile_pool(name="mapp", bufs=2) as mpool,
            tc.tile_pool(name="wp", bufs=5) as wpool,
            tc.tile_pool(name="ps", bufs=4, space="PSUM") as pspool,
        ):
            stage = constp.tile([128, nslots], out_dt)
            wmax = max(max(gen_ltot), 8)

            def emit_map_chunk(map_t, g, eng):
                r0, r1 = seg_off[g], seg_off[g] + steps[g]
                if r1 > r0:
                    for q in range(NQ):
                        base = q * srows
                        eng.dma_start(
                            out=map_t[:, (base + r0) * C:(base + r1) * C],
                            in_=mp[:, (base + r0) * C:(base + r1) * C])

            for _rep in range(rep):
                map_t = mpool.tile([KR, NQ * srows * C], dt, tag="map")
                # map chunks alternate SP/ACT rings, interleaved with the
                # per-gen W loads so gen g's W isn't queued behind the whole
                # map on one ring
                wcols_w = min(G, steps[0] * C)
                if WARM > 0:
                    # land the warm-up operand columns first so the PE starts
                    # its clock-gate-tripping burst as early as possible
                    nc.sync.dma_start(out=map_t[:, 0:wcols_w],
                                      in_=mp[:, 0:wcols_w])
                    wm = pspool.tile([128, G], mybir.dt.float32, tag="warm")
                    for _ in range(WARM):
                        nc.tensor.matmul(wm[:, :wcols_w], map_t[:, 0:C],
                                         map_t[:, 0:wcols_w],
                                         start=True, stop=True)
                    r1 = (seg_off[0] + steps[0]) * C
                    if r1 > wcols_w:
                        nc.sync.dma_start(out=map_t[:, wcols_w:r1],
                                          in_=mp[:, wcols_w:r1])
                    for q in range(1, NQ):
                        base = q * srows
                        nc.sync.dma_start(
                            out=map_t[:, base * C:(base + steps[0]) * C],
                            in_=mp[:, base * C:(base + steps[0]) * C])
                else:
                    emit_map_chunk(map_t, 0, nc.sync)
                ps = None
                w_t = None
                cur_g = -1
                for (g, s, q, cl, ch, col0, first, last) in sched:
                    if g != cur_g:
                        ps = pspool.tile([128, G], mybir.dt.float32, tag="ps")
                        w_t = wpool.tile([KR, wmax], dt, tag="wt")
                        wo = gen_woff[g]
                        lg = gen_ltot[g]
                        if lg > 0:
                            # W on the ring opposite the next map chunk
                            weng = nc.sync if g % 2 else nc.scalar
                            weng.dma_start(out=w_t[:, :lg],
                                           in_=w[:, wo:wo + lg])
                        if g + 1 < ngens:
                            emit_map_chunk(map_t, g + 1,
                                           nc.scalar if (g + 1) % 2 else nc.sync)
                        cur_g = g
                    row0 = (q * srows + seg_off[g] + s) * C
                    lhsT = map_t[:, row0:row0 + C]
                    rhs = w_t[:, col0:col0 + (ch - cl)]
                    nc.tensor.matmul(ps[:, cl:ch], lhsT, rhs,
                                     start=first, stop=last,
                                     tile_position=(0, 0))
                    if last:
                        gc = meta["gen_cols"][g]
                        c0 = meta["coff"][g]
                        nc.vector.tensor_copy(stage[:, c0:c0 + gc],
                                              ps[:, :gc])
                        # batched output drains: bulk after the second-to-last
                        # gen, remainder (small tail) after the last
                        if g == ngens - 2:
                            nc.sync.dma_start(out=o[:, :c0 + gc],
                                              in_=stage[:, :c0 + gc])
                        elif g == ngens - 1:
                            lo = meta["coff"][g]
                            nc.sync.dma_start(out=o[:, lo:lo + gc],
                                              in_=stage[:, lo:lo + gc])

    _split_drains(nc, mybir, bass_rust)
    nc.finalize()
    return nc


_prog_cache = {}


def _get_program(meta, rep=1):
    key = (meta["sched"], meta["rstar"], meta["nslots"], rep, DT_MODE)
    if key not in _prog_cache:
        _prog_cache[key] = _build_program(meta, rep=rep)
    return _prog_cache[key]


def _run(nc, in_maps):
    from concourse.bass_utils import run_bass_kernel_spmd
    last_err = None
    for _attempt in range(3):
        try:
            res = run_bass_kernel_spmd(nc, in_maps, list(range(NCORES)))
            return res.results
        except Exception as e:  # transient device wedge -> retry
            last_err = e
            time.sleep(2.0)
    raise last_err


# ----------------------------------------------------------------------------
# public entry
# ----------------------------------------------------------------------------

def kernel(data, rois, offset):
    data = np.asarray(data, f32)
    rois = np.asarray(rois, f32)
    offset = np.asarray(offset, f32)
    N = rois.shape[0]

    plan = _plan(rois, offset)
    if len(plan["meta"]["sched"]) == 0:   # every bin fully masked
        return np.zeros((N, C, POOLED, POOLED), f32)
    in_maps = _build_inputs(plan, data)
    nc = _get_program(plan["meta"])
    results = _run(nc, in_maps)

    flat = np.zeros((N * POOLED * POOLED, C), f32)   # [bin, c]
    for ci in range(NCORES):
        sbin = plan["slot_bin"][ci]
        m = sbin >= 0
        if not m.any():
            continue
        sb = np.asarray(results[ci]["o"], f32)  # [128, nslots]
        # straddling bins occupy one slot per active quad; sum the parts
        np.add.at(flat, sbin[m], sb.T[m])
    flat[~plan["ok"]] = 0.0   # degenerate bins never touched on device
    out = flat.reshape(N, POOLED, POOLED, C).transpose(0, 3, 1, 2)
    return np.ascontiguousarray(out)


# revision 91
# speedup vs baseline: 1.0060x; 1.0060x over previous
"""Deformable PSROI pooling (group_size=1, num_classes=1) on 8 trn2 NeuronCores.

Strategy ("map sweep"):
  out[n, c, ph, pw] = sum_{y,x} KY[bin, y] * KX[bin, x] * data[b, c, y, x]
where KX/KY are per-bin bilinear "hat" weight profiles (sums over the 4x4
sample grid, with sample masks and 1/count folded in).  Each core holds a
slice of one batch's feature map in SBUF in [x(partitions), (y, c)] layout;
for each feature row y it issues one fp16 TensorE matmul
    psum[c, bins] += map_row[x, c].T @ W_y[x, bins]
accumulating bins in PSUM 512-column "generations" (bins sorted by first
active row), with host-precomputed W streamed per generation.

Everything is fp16 (weights, map, staged output; fp32 PSUM accumulate),
which quarters the fp32 PE pass count and halves HBM traffic.  A short
burst of dummy matmuls at the start of each rep keeps ~3.4us of continuous
PE activity so the HAM clock gate lifts the PE from 1.2GHz to 2.4GHz before
the real stream begins (worth ~1.5x end to end).

Sharding: bins are split by (batch, ylo-quantile) into 8 equal-count shards
(quantile keeps each core's map read to a ~48-row window).  The compiled
program is shared by all cores; all per-core variation lives in the input
tensors (map slice, W stream).  Cross-core schedule alignment is
per-generation, with the map supplied as per-generation row segments so each
core anchors a generation at its own starting row; matmul column windows are
the cross-core envelope, with zero weights padding inactive slots.

Probed dead ends kept for the record: switching PE tile_position between
matmuls hangs the exec unit with this toolchain, and the (64,128) row-tiled
mode runs ~2x slower per matmul, so narrow-K x-window compression of W
(quad/half streams) loses to the dense K=128 sweep even though it moves
~2.5x fewer W bytes.  NQ/WW plumbing for those variants remains (KNQ env).
"""
import os
import sys
import time

import numpy as np

sys.path.insert(0, "/opt/trn_rl_repo")

SPATIAL_SCALE = np.float32(0.0625)
POOLED = 7
SAMPLES = 4
TRANS_STD = np.float32(0.1)
B, C, H, W = 2, 128, 128, 128
NCORES = 8
GEN_COLS = 512
# "f32" (exact, 4-pass PE), "f16" (1-pass, ~1e-3, half the DMA bytes)
DT_MODE = os.environ.get("KDT", "f16")

f32 = np.float32
YSENT = 10 ** 6  # sentinel ylo for bins with all-zero weights
# x-windows: two 64-row halves [0,64) and [64,128), both presented to the PE
# at partitions [0,64) (the map tile holds the upper half relocated), so every
# matmul is a uniform K=64 tile_size=(64,128) at tile_position (0,0).
# Switching tile_position between matmuls hangs the exec unit (probed), so a
# single uniform tile is required.
NQ = int(os.environ.get("KNQ", "1"))
WW = 128 // NQ
# contraction uses KR partitions (one always-zero pad row when WW<128):
# partition size >64 rounds the PE tile_size up to the standard (128,128)
# mode.  The (64,128) row-tiled mode measured ~2x slower per matmul (weight
# load no longer overlaps), and switching tile_position between matmuls
# hangs the exec unit.
KR = min(WW + 1, 128)
WIDEN = os.environ.get("KWIDEN", "0") == "1"
# dummy matmuls at rep start: ~3.4us of continuous PE activity trips the HAM
# clock gate from 1.2GHz to 2.4GHz before the real stream begins
WARM = int(os.environ.get("KWARM", "8"))


# ----------------------------------------------------------------------------
# host planning
# ----------------------------------------------------------------------------

def _bin_params(rois, offset):
    """Exact float32 emulation of the reference coordinate math.

    Returns per-bin (N*49) arrays: batch, dense hat profiles kx/ky [nb, 128]
    (ky has 1/count folded in), y-support [ylo, yhi], x-support class.
    """
    N = rois.shape[0]
    P, S = POOLED, SAMPLES
    rois = rois.astype(f32)
    offset = offset.astype(f32)

    batch_ind = rois[:, 0].astype(np.int32)
    roi_sw = np.round(rois[:, 1]) * SPATIAL_SCALE - f32(0.5)
    roi_sh = np.round(rois[:, 2]) * SPATIAL_SCALE - f32(0.5)
    roi_ew = np.round(rois[:, 3] + f32(1.0)) * SPATIAL_SCALE - f32(0.5)
    roi_eh = np.round(rois[:, 4] + f32(1.0)) * SPATIAL_SCALE - f32(0.5)
    roi_w = np.maximum(roi_ew - roi_sw, f32(0.1))
    roi_h = np.maximum(roi_eh - roi_sh, f32(0.1))
    bin_w = roi_w / f32(P)
    bin_h = roi_h / f32(P)
    sub_w = bin_w / f32(S)
    sub_h = bin_h / f32(S)

    pidx = np.arange(P, dtype=f32)
    trans_x = offset[:, 0] * TRANS_STD          # [N, 7(ph), 7(pw)]
    trans_y = offset[:, 1] * TRANS_STD
    pw = pidx[None, None, :]
    ph = pidx[None, :, None]
    wstart = pw * bin_w[:, None, None] + roi_sw[:, None, None] + trans_x * roi_w[:, None, None]
    hstart = ph * bin_h[:, None, None] + roi_sh[:, None, None] + trans_y * roi_h[:, None, None]

    sidx = np.arange(S, dtype=f32)
    w_s = wstart[..., None] + sidx * sub_w[:, None, None, None]     # [N,7,7,4]
    h_s = hstart[..., None] + sidx * sub_h[:, None, None, None]
    mask_w = (w_s >= f32(-0.5)) & (w_s <= f32(W) - f32(0.5))
    mask_h = (h_s >= f32(-0.5)) & (h_s <= f32(H) - f32(0.5))
    wc = np.clip(w_s, f32(0.0), f32(W - 1))
    hc = np.clip(h_s, f32(0.0), f32(H - 1))

    cnt = (mask_h.sum(-1) * mask_w.sum(-1)).astype(f32)             # [N,7,7]
    inv = np.where(cnt > 0, f32(1.0) / np.maximum(cnt, f32(1.0)), f32(0.0))

    nb = N * P * P
    wc = wc.reshape(nb, S)
    hc = hc.reshape(nb, S)
    mask_w = mask_w.reshape(nb, S)
    mask_h = mask_h.reshape(nb, S)
    inv = inv.reshape(nb)

    xg = np.arange(W, dtype=np.float64)
    kx = np.zeros((nb, W), np.float64)
    ky = np.zeros((nb, H), np.float64)
    for s in range(S):
        kx += mask_w[:, s, None] * np.maximum(0.0, 1.0 - np.abs(wc[:, s, None].astype(np.float64) - xg))
        ky += mask_h[:, s, None] * np.maximum(0.0, 1.0 - np.abs(hc[:, s, None].astype(np.float64) - xg))
    ky *= inv[:, None]
    kx = kx.astype(f32)
    ky = ky.astype(f32)

    ky_nz = ky != 0
    has_y = ky_nz.any(axis=1)
    ylo = np.where(has_y, ky_nz.argmax(axis=1), YSENT).astype(np.int64)
    yhi = np.where(has_y, H - 1 - ky_nz[:, ::-1].argmax(axis=1), -YSENT).astype(np.int64)

    kx_nz = kx != 0
    has_x = kx_nz.any(axis=1)
    xlo = np.where(has_x, kx_nz.argmax(axis=1), 0).astype(np.int64)
    xhi = np.where(has_x, W - 1 - kx_nz[:, ::-1].argmax(axis=1), -1).astype(np.int64)

    ok = has_y & has_x
    ylo = np.where(ok, ylo, YSENT)
    yhi = np.where(ok, yhi, -YSENT)
    # per-bin active x-windows: window q active iff kx[WW*q : WW*(q+1)] != 0
    qact = np.zeros((nb, NQ), bool)
    for q in range(NQ):
        qact[:, q] = ok & (xlo <= WW * q + WW - 1) & (xhi >= WW * q)

    batch = np.repeat(batch_ind, P * P)
    return batch, kx, ky, ylo, yhi, ok, qact


def _plan(rois, offset):
    batch, kx, ky, ylo, yhi, ok, qact = _bin_params(rois, offset)
    G = GEN_COLS

    # shard bins: (batch, ylo-quantile) -> 8 shards with equal-ish counts
    shards = []
    for b in range(B):
        ids = np.where(batch == b)[0]
        ids = ids[np.lexsort((yhi[ids], ylo[ids]))]
        q = NCORES // B
        shards.extend(ids[int(len(ids) * i / q):int(len(ids) * (i + 1) / q)]
                      for i in range(q))
    assert len(shards) == NCORES

    # expand bins -> (bin, quad) pairs; a straddling bin occupies one slot in
    # each of its (at most 2) active quads, summed on the host at unshard.
    # Pairs are kept in (ylo, yhi) order within each quad queue.
    pair_q_lists = []   # [ci][q] -> bin ids in (ylo, yhi) order
    for ci in range(NCORES):
        ids = shards[ci]
        pair_q_lists.append([ids[qact[ids, q]] for q in range(NQ)])

    # per-shard absolute row window
    row_start = np.zeros(NCORES, np.int64)
    extents = []
    for ids in shards:
        real = ids[ylo[ids] < YSENT]
        if len(real):
            extents.append((int(ylo[real].min()), int(yhi[real].max())))
        else:
            extents.append((0, 0))
    rstar = max(b_ - a_ + 1 for a_, b_ in extents)
    rstar = min(H, -(-rstar // 8) * 8)
    # anchor at each core's own extent start (map rows past H are zero-padded)
    # so per-gen windows align across cores in relative-row space
    for ci, (a_, b_) in enumerate(extents):
        row_start[ci] = a_

    # generation layout with SHARED window-run offsets.  Each core's pairs
    # form one (ylo, yhi)-sorted stream; all cores advance in lockstep by k
    # pairs per gen — same stream index means the same quantile position, so
    # window profiles align across cores once each (core, gen) re-anchors at
    # its own first row (the map is supplied as per-gen row segments).
    # Per (gen, window) capacity R_gq = cross-core max of pairs taken; cores
    # with fewer pad with dead slots.
    pair_bin, pair_quad, qcums = [], [], []
    for ci in range(NCORES):
        ids = shards[ci]
        pb = np.repeat(ids, qact[ids].sum(axis=1))
        pq = np.concatenate([np.nonzero(qact[i])[0] for i in ids]) \
            if len(ids) else np.zeros(0, np.int64)
        pair_bin.append(pb.astype(np.int64))
        pair_quad.append(pq.astype(np.int64))
        qcums.append(np.concatenate(
            [np.zeros((NQ, 1), np.int64),
             np.cumsum(pq[None, :] == np.arange(NQ)[:, None], axis=1)], axis=1)
            if len(pq) else np.zeros((NQ, 1), np.int64))
    totals = np.array([len(p) for p in pair_bin], np.int64)
    ptr = np.zeros(NCORES, np.int64)
    gen_caps = []       # [g][q] -> R_gq
    gen_k = []          # [g] -> pairs taken per core

    def quad_counts(k):
        n = np.zeros((NCORES, NQ), np.int64)
        for ci in range(NCORES):
            e = min(int(ptr[ci]) + k, int(totals[ci]))
            n[ci] = qcums[ci][:, e] - qcums[ci][:, ptr[ci]]
        return n

    while (ptr < totals).any():
        lo_k, hi_k = 1, int((totals - ptr).max())
        while lo_k < hi_k:     # largest k with sum_q max_ci counts <= G
            mid = (lo_k + hi_k + 1) // 2
            if quad_counts(mid).max(axis=0).sum() <= G:
                lo_k = mid
            else:
                hi_k = mid - 1
        n = quad_counts(lo_k)
        gen_caps.append([int(n[:, q].max()) for q in range(NQ)])
        gen_k.append(lo_k)
        ptr = np.minimum(ptr + lo_k, totals)
    ngens = len(gen_caps)
    gen_cols = [sum(R) for R in gen_caps]
    coff = np.concatenate([[0], np.cumsum(gen_cols)]).astype(np.int64)
    nslots = int(coff[-1])

    # per-(core, gen) row anchor + per-gen map segments
    t0 = np.zeros((NCORES, ngens), np.int64)
    span = np.zeros((NCORES, ngens), np.int64)
    runs = {}
    slot_bin = []      # per core: bin id for each slot, -1 for pad slots
    for ci in range(NCORES):
        rs = int(row_start[ci])
        p = 0
        slots = np.full(nslots, -1, np.int64)
        for g, R in enumerate(gen_caps):
            e = min(p + gen_k[g], int(totals[ci]))
            gb = pair_bin[ci][p:e]
            gq = pair_quad[ci][p:e]
            p = e
            if len(gb) == 0:
                continue
            real = gb[ylo[gb] < YSENT]
            if len(real) == 0:
                continue
            a_ = int(ylo[real].min()) - rs
            t0[ci, g] = a_
            span[ci, g] = int(yhi[real].max()) - rs - a_ + 1
            roff = 0
            for q in range(NQ):
                m = gq == q
                take = int(m.sum())
                if take > 0:
                    sel = gb[m]
                    slots[coff[g] + roff:coff[g] + roff + take] = sel
                    runs[(ci, g, q)] = (roff, ylo[sel] - rs - a_,
                                        yhi[sel] - rs - a_, take)
                roff += R[q]
        slot_bin.append(slots)
    steps = [int(span[:, g].max()) for g in range(ngens)]
    seg_off = np.concatenate([[0], np.cumsum(steps)]).astype(np.int64)
    srows = int(seg_off[-1])

    # shared schedule: per (gen, step, quad) the column envelope and the W
    # stream cursor.  W layout per gen: the 4 quad streams stacked in
    # partition quarters over one rectangular [128, Lg] block (one DMA).
    sched = []          # (g, s, q, cl, ch, col0, first, last)
    gen_ltot = []
    for g, R in enumerate(gen_caps):
        S_g = steps[g]
        qoff = np.concatenate([[0], np.cumsum(R)]).astype(np.int64)
        cur = 0
        entries = []
        widened = set()
        for s in range(S_g):
            for q in range(NQ):
                cl, ch = YSENT, 0
                for ci in range(NCORES):
                    r = runs.get((ci, g, q))
                    if r is None:
                        continue
                    roff, yl, yh, take = r
                    # shared step: row rel T[g]+s for every core
                    hi = int(np.count_nonzero(yl <= s))
                    live = np.nonzero(yh >= s)[0]
                    lo = int(live[0]) if len(live) else hi
                    if hi > lo:
                        cl = min(cl, roff + lo)
                        ch = max(ch, roff + hi)
                if ch > cl:
                    if WIDEN and q not in widened:
                        # first entry of this window covers the whole run so
                        # every slot (incl. pads) is written exactly once
                        # (pad slots otherwise hold stale PSUM, masked on the
                        # host; only the strict simulator needs the cover)
                        widened.add(q)
                        cl, ch = int(qoff[q]), int(qoff[q + 1])
                    entries.append((g, s, q, cl, ch, cur))
                    cur += ch - cl
        gen_ltot.append(cur)
        for i, e in enumerate(entries):
            sched.append(e + (i == 0, i == len(entries) - 1))
    gen_woff = np.concatenate([[0], np.cumsum(gen_ltot)]).astype(np.int64)
    wtot = int(gen_woff[-1])

    meta = dict(rstar=int(rstar), ngens=int(ngens), nslots=int(nslots),
                sched=tuple(sched), wtot=wtot, srows=srows,
                steps=tuple(steps),
                seg_off=tuple(int(x) for x in seg_off),
                gen_ltot=tuple(gen_ltot), gen_cols=tuple(gen_cols),
                coff=tuple(int(x) for x in coff),
                gen_woff=tuple(int(x) for x in gen_woff))
    return dict(meta=meta, slot_bin=slot_bin, runs=runs, t0=t0,
                row_start=row_start,
                kx=kx, ky=ky, ylo=ylo, ok=ok)


def _build_inputs(plan, data):
    meta = plan["meta"]
    sched, srows, wtot = meta["sched"], meta["srows"], meta["wtot"]
    gen_woff, coff, seg_off = meta["gen_woff"], meta["coff"], meta["seg_off"]
    kx, ky, runs = plan["kx"], plan["ky"], plan["runs"]
    t0 = plan["t0"]
    data_perm = np.ascontiguousarray(data.transpose(0, 3, 2, 1))  # [B, W(x), H(y), C]

    np_dt = np.float16 if DT_MODE == "f16" else f32
    in_maps = []
    for ci in range(NCORES):
        sbin = plan["slot_bin"][ci]
        b = ci // (NCORES // B)
        rs = int(plan["row_start"][ci])
        # per-gen map row segments; window q's x-rows at partitions [0, WW)
        mp = np.zeros((KR, NQ, srows, C), np_dt)
        for g, S_g in enumerate(meta["steps"]):
            if S_g == 0:
                continue
            y0 = rs + int(t0[ci, g])
            y1 = min(H, y0 + S_g)
            if y1 <= y0:
                continue
            for q in range(NQ):
                mp[:WW, q, seg_off[g]:seg_off[g] + (y1 - y0), :] = \
                    data_perm[b, WW * q:WW * (q + 1), y0:y1, :]
        wbuf = np.zeros((KR, max(wtot, 8)), np_dt)
        for (g, s, q, cl, ch, col0, first, last) in sched:
            r = runs.get((ci, g, q))
            if r is None:
                continue
            y = rs + int(t0[ci, g]) + s
            if y >= H:
                continue
            roff, yl, yh, take = r
            # only this core's own window-q run; envelope overhang stays zero
            j0, j1 = max(cl, roff), min(ch, roff + take)
            if j1 <= j0:
                continue
            sel = sbin[coff[g] + j0:coff[g] + j1]
            vals = kx[sel, WW * q:WW * (q + 1)] * ky[sel, y][:, None]  # [n, WW]
            c0 = gen_woff[g] + col0 + (j0 - cl)
            wbuf[:WW, c0:c0 + (j1 - j0)] = vals.T
        in_maps.append({"mp": np.ascontiguousarray(mp.reshape(KR, NQ * srows * C)),
                        "w": wbuf})
    return in_maps


# ----------------------------------------------------------------------------
# device program
# ----------------------------------------------------------------------------

def _split_drains(nc, mybir, bass_rust):
    for f_ in nc.m.functions:
        for blk in f_.blocks:
            newlist = []
            for ins in blk.instructions:
                wts = list(ins.sync_info.on_wait) if ins.sync_info else []
                if len(wts) > 1 and type(ins).__name__ == "InstDrain":
                    for j, wx in enumerate(wts[1:]):
                        nop = mybir.InstNoOp(name=f"splitw_{id(ins)}_{j}", ins=[], outs=[])
                        nop.engine = ins.engine
                        nop.sync_info = bass_rust.SyncInfo(on_wait=[wx], on_update=[])
                        newlist.append(nop)
                    ins.sync_info.on_wait = wts[:1]
                newlist.append(ins)
            blk.instructions = newlist


def _build_program(meta, rep=1):
    import concourse.bacc as bacc
    import concourse.mybir as mybir
    import bass_rust
    from concourse.tile import TileContext

    ngens, nslots = meta["ngens"], meta["nslots"]
    sched, wtot, srows = meta["sched"], meta["wtot"], meta["srows"]
    steps, seg_off = meta["steps"], meta["seg_off"]
    gen_woff, gen_ltot = meta["gen_woff"], meta["gen_ltot"]
    dt = mybir.dt.float16 if DT_MODE == "f16" else mybir.dt.float32
    out_dt = mybir.dt.float16 if DT_MODE == "f16" else mybir.dt.float32
    G = GEN_COLS

    nc = bacc.Bacc()
    mp = nc.declare_dram_parameter("mp", [KR, NQ * srows * C], dt, isOutput=False)
    w = nc.declare_dram_parameter("w", [KR, max(wtot, 8)], dt, isOutput=False)
    o = nc.declare_dram_parameter("o", [128, nslots], out_dt, isOutput=True)

    with TileContext(nc) as tc:
        with (
            tc.tile_pool(name="const", bufs=1) as constp,
            tc.tile_pool(name="mapp", bufs=2) as mpool,
            tc.tile_pool(name="wp", bufs=5) as wpool,
            tc.tile_pool(name="ps", bufs=4, space="PSUM") as pspool,
        ):
            stage = constp.tile([128, nslots], out_dt)
            wmax = max(max(gen_ltot), 8)

            def emit_map_chunk(map_t, g, eng):
                r0, r1 = seg_off[g], seg_off[g] + steps[g]
                if r1 > r0:
                    for q in range(NQ):
                        base = q * srows
                        eng.dma_start(
                            out=map_t[:, (base + r0) * C:(base + r1) * C],
                            in_=mp[:, (base + r0) * C:(base + r1) * C])

            for _rep in range(rep):
                map_t = mpool.tile([KR, NQ * srows * C], dt, tag="map")
                # map chunks alternate SP/ACT rings, interleaved with the
                # per-gen W loads so gen g's W isn't queued behind the whole
                # map on one ring
                emit_map_chunk(map_t, 0, nc.sync)
                if WARM > 0:
                    wm = pspool.tile([128, G], mybir.dt.float32, tag="warm")
                    wcols_w = min(G, steps[0] * C)
                    for _ in range(WARM):
                        nc.tensor.matmul(wm[:, :wcols_w], map_t[:, 0:C],
                                         map_t[:, 0:wcols_w],
                                         start=True, stop=True)
                ps = None
                w_t = None
                cur_g = -1
                for (g, s, q, cl, ch, col0, first, last) in sched:
                    if g != cur_g:
                        ps = pspool.tile([128, G], mybir.dt.float32, tag="ps")
                        w_t = wpool.tile([KR, wmax], dt, tag="wt")
                        wo = gen_woff[g]
                        lg = gen_ltot[g]
                        if lg > 0:
                            # W on the ring opposite the next map chunk
                            weng = nc.sync if g % 2 else nc.scalar
                            weng.dma_start(out=w_t[:, :lg],
                                           in_=w[:, wo:wo + lg])
                        if g + 1 < ngens:
                            emit_map_chunk(map_t, g + 1,
                                           nc.scalar if (g + 1) % 2 else nc.sync)
                        cur_g = g
                    row0 = (q * srows + seg_off[g] + s) * C
                    lhsT = map_t[:, row0:row0 + C]
                    rhs = w_t[:, col0:col0 + (ch - cl)]
                    nc.tensor.matmul(ps[:, cl:ch], lhsT, rhs,
                                     start=first, stop=last,
                                     tile_position=(0, 0))
                    if last:
                        gc = meta["gen_cols"][g]
                        c0 = meta["coff"][g]
                        nc.vector.tensor_copy(stage[:, c0:c0 + gc],
                                              ps[:, :gc])
                        # batched output drains: bulk after the second-to-last
                        # gen, remainder (small tail) after the last
                        if g == ngens - 2:
                            nc.sync.dma_start(out=o[:, :c0 + gc],
                                              in_=stage[:, :c0 + gc])
                        elif g == ngens - 1:
                            lo = meta["coff"][g]
                            nc.sync.dma_start(out=o[:, lo:lo + gc],
                                              in_=stage[:, lo:lo + gc])

    _split_drains(nc, mybir, bass_rust)
    nc.finalize()
    return nc


_prog_cache = {}


def _get_program(meta, rep=1):
    key = (meta["sched"], meta["rstar"], meta["nslots"], rep, DT_MODE)
    if key not in _prog_cache:
        _prog_cache[key] = _build_program(meta, rep=rep)
    return _prog_cache[key]


def _run(nc, in_maps):
    from concourse.bass_utils import run_bass_kernel_spmd
    last_err = None
    for _attempt in range(3):
        try:
            res = run_bass_kernel_spmd(nc, in_maps, list(range(NCORES)))
            return res.results
        except Exception as e:  # transient device wedge -> retry
            last_err = e
            time.sleep(2.0)
    raise last_err


# ----------------------------------------------------------------------------
# public entry
# ----------------------------------------------------------------------------

def kernel(data, rois, offset):
    data = np.asarray(data, f32)
    rois = np.asarray(rois, f32)
    offset = np.asarray(offset, f32)
    N = rois.shape[0]

    plan = _plan(rois, offset)
    if len(plan["meta"]["sched"]) == 0:   # every bin fully masked
        return np.zeros((N, C, POOLED, POOLED), f32)
    in_maps = _build_inputs(plan, data)
    nc = _get_program(plan["meta"])
    results = _run(nc, in_maps)

    flat = np.zeros((N * POOLED * POOLED, C), f32)   # [bin, c]
    for ci in range(NCORES):
        sbin = plan["slot_bin"][ci]
        m = sbin >= 0
        if not m.any():
            continue
        sb = np.asarray(results[ci]["o"], f32)  # [128, nslots]
        # straddling bins occupy one slot per active quad; sum the parts
        np.add.at(flat, sbin[m], sb.T[m])
    flat[~plan["ok"]] = 0.0   # degenerate bins never touched on device
    out = flat.reshape(N, POOLED, POOLED, C).transpose(0, 3, 1, 2)
    return np.ascontiguousarray(out)
